# revision 24
# baseline (speedup 1.0000x reference)
"""Trainium2 Bass kernel for EnhancedAttention (B=2, S=2048, DM=1024, H=16, D=64).

Strategy: 8 NeuronCores = data-parallel over batch (2) x tensor-parallel over
heads (4 heads/core). Each core runs a fused QKV-projection + attention +
output-projection program; the host sums the 4 output-projection partials per
batch element and applies the biases.

Schedule (v2): four attention blocks (qh, g) in qh-major order, each block
qq-major (all 16 key tiles for query half 0, then half 1). Projection and
final-projection pieces are interleaved as per-bundle "extras" so the PE never
starves while the activation engine streams the exps. Softmax denominators
come out of the PV matmul itself: each head's V tile is [v(64) | ones(64)], so
PSUM rows 64-127 hold sum(exp) already broadcast across 64 partitions and the
normalize is a DVE reciprocal+multiply with no DMA round-trip. QT/KT/es/VA are
bf16 (same PE rate, half the SBUF); accumulation stays fp32.

kernel(**inputs) takes the full unsharded inputs and returns the full output.
"""

import os
import sys

for _p in ("/opt/trn_rl_repo", "/root/.axon_site/_ro/trn_rl_repo"):
    if os.path.isdir(_p) and _p not in sys.path:
        sys.path.append(_p)

import numpy as np
from contextlib import ExitStack

import concourse.bass as bass
import concourse.mybir as mybir
import concourse.tile as tile
from concourse import bacc
from concourse.bass import ts, ds

F32 = mybir.dt.float32
F32R = mybir.dt.float32r
BF16 = mybir.dt.bfloat16
EXP = mybir.ActivationFunctionType.Exp
COPY = mybir.ActivationFunctionType.Copy
ADD = mybir.AluOpType.add
MULT = mybir.AluOpType.mult

S = 2048
DM = 1024
HD = 64
PV_LAG = 5


class _Rep:
    """Per-repetition tile state."""

    __slots__ = ("xts", "w_sb", "bqk_sb", "pos_sb", "wo_sb",
                 "QT", "KT", "OT", "VA", "pv_queue")

    def __init__(self):
        self.xts = [[None] * 8 for _ in range(4)]
        self.w_sb = {}
        self.QT = [[None] * 4 for _ in range(2)]   # [g][qb]
        self.KT = [[None] * 4 for _ in range(2)]   # [g][qb]
        self.OT = [None, None]                     # [g]
        self.VA = [None] * 16
        self.pv_queue = []


def build_program(use_f32r=True, repeat=1):
    MMDT = F32R if use_f32r else F32
    nc = bacc.Bacc("TRN2", debug=False)
    xt = nc.dram_tensor("xt", [DM, S], MMDT, kind="ExternalInput").ap()
    wq = nc.dram_tensor("wq", [DM, 256], MMDT, kind="ExternalInput").ap()
    wk = nc.dram_tensor("wk", [DM, 256], MMDT, kind="ExternalInput").ap()
    wv = nc.dram_tensor("wv", [DM, 256], MMDT, kind="ExternalInput").ap()
    bqk = nc.dram_tensor("bqk", [128, 4], F32, kind="ExternalInput").ap()
    post = nc.dram_tensor("post", [128, S], F32, kind="ExternalInput").ap()
    wo = nc.dram_tensor("wo", [256, DM], MMDT, kind="ExternalInput").ap()
    out = nc.dram_tensor("out", [S, DM], F32, kind="ExternalOutput").ap()

    with tile.TileContext(nc) as tc, ExitStack() as ctx:
        p_xt = ctx.enter_context(tc.tile_pool(name="xt", bufs=32))
        p_w = ctx.enter_context(tc.tile_pool(name="w", bufs=24))
        p_sing = ctx.enter_context(tc.tile_pool(name="sing", bufs=2))
        p_wo = ctx.enter_context(tc.tile_pool(name="wo", bufs=4))
        p_qt = ctx.enter_context(tc.tile_pool(name="qt", bufs=8))
        p_kt = ctx.enter_context(tc.tile_pool(name="kt", bufs=8))
        p_va = ctx.enter_context(tc.tile_pool(name="va", bufs=16))
        p_es = ctx.enter_context(tc.tile_pool(name="es", bufs=8))
        p_ot = ctx.enter_context(tc.tile_pool(name="ot", bufs=2))
        p_rc = ctx.enter_context(tc.tile_pool(name="rc", bufs=2))
        p_fo = ctx.enter_context(tc.tile_pool(name="fo", bufs=3))
        ps_b = ctx.enter_context(tc.tile_pool(name="psb", bufs=2, space="PSUM"))
        ps_o = ctx.enter_context(tc.tile_pool(name="pso", bufs=2, space="PSUM"))

        reps = [_Rep() for _ in range(repeat)]

        # ---- DMA emission ------------------------------------------------
        def dmas(r):
            st = reps[r]
            st.bqk_sb = p_sing.tile([128, 4], F32, tag="bqk", name=f"bqk{r}")
            nc.gpsimd.dma_start(out=st.bqk_sb, in_=bqk)
            st.pos_sb = p_sing.tile([128, S], F32, tag="post", name=f"pos{r}")
            nc.gpsimd.dma_start(out=st.pos_sb, in_=post)

            def dma_w(nm, dram):
                tiles = []
                for t in range(8):
                    w_t = p_w.tile([128, 256], MMDT, tag="w",
                                   name=f"w{nm}{r}_{t}")
                    nc.sync.dma_start(out=w_t, in_=dram[ts(t, 128), :])
                    tiles.append(w_t)
                st.w_sb[nm] = tiles

            def dma_xt(qb):
                for t in range(8):
                    x_t = p_xt.tile([128, 512], MMDT, tag="xt",
                                    name=f"x_t{r}_{qb}_{t}")
                    nc.sync.dma_start(out=x_t, in_=xt[ts(t, 128), ts(qb, 512)])
                    st.xts[qb][t] = x_t

            dma_w("k", wk)
            dma_w("q", wq)
            dma_xt(0)
            dma_w("v", wv)
            dma_xt(1)
            dma_xt(2)
            dma_xt(3)
            st.wo_sb = []
            for t in range(2):
                wo_t = p_wo.tile([128, DM], MMDT, tag="wo", name=f"wo{r}_{t}")
                nc.sync.dma_start(out=wo_t, in_=wo[ts(t, 128), :])
                st.wo_sb.append(wo_t)

        # ---- projection pieces ------------------------------------------
        def kq_piece(r, g, qb, which):
            def f():
                st = reps[r]
                if which == 0:
                    psk = ps_b.tile([128, 512], F32, tag="psb",
                                    name=f"psk{r}_{g}{qb}")
                    for t in range(8):
                        nc.tensor.matmul(
                            psk, st.w_sb["k"][t][:, ds(g * 128, 128)],
                            st.xts[qb][t], start=(t == 0), stop=(t == 7))
                    kt_t = p_kt.tile([128, 512], BF16, tag="kt",
                                     name=f"KT{r}_{g}_{qb}")
                    nc.vector.scalar_tensor_tensor(
                        out=kt_t, in0=psk, scalar=st.bqk_sb[:, ds(2 + g, 1)],
                        in1=st.pos_sb[:, ts(qb, 512)], op0=ADD, op1=ADD)
                    st.KT[g][qb] = kt_t
                else:
                    psq = ps_b.tile([128, 512], F32, tag="psb",
                                    name=f"psq{r}_{g}{qb}")
                    for t in range(8):
                        nc.tensor.matmul(
                            psq, st.w_sb["q"][t][:, ds(g * 128, 128)],
                            st.xts[qb][t], start=(t == 0), stop=(t == 7))
                    qt_t = p_qt.tile([128, 512], BF16, tag="qt",
                                     name=f"QT{r}_{g}_{qb}")
                    nc.vector.tensor_scalar_add(
                        qt_t, psq, st.bqk_sb[:, ds(g, 1)])
                    st.QT[g][qb] = qt_t
            return f

        def v_piece(r, stt):
            def f():
                st = reps[r]
                va = p_va.tile([128, 512], BF16, tag="va", name=f"va{r}_{stt}")
                va_r = va.rearrange("p (h c) -> p h c", h=4)
                # ones half-block per head, emitted first so the DVE write
                # overlaps the projection matmuls
                nc.vector.memset(va_r[:, :, 64:128], 1.0)
                psv = ps_b.tile([128, 256], F32, tag="psb",
                                name=f"psv{r}_{stt}")
                for t in range(8):
                    nc.tensor.matmul(
                        psv, st.xts[stt // 4][t][:, ds((stt % 4) * 128, 128)],
                        st.w_sb["v"][t], start=(t == 0), stop=(t == 7))
                psv_r = psv.rearrange("p (h d) -> p h d", h=4)
                nc.vector.tensor_copy(va_r[:, :, 0:64], psv_r)
                st.VA[stt] = va
            return f

        def fin_piece(r, qt_i, pool=None, split=False):
            def f():
                st = reps[r]
                pl, tg = (ps_o, "otp") if pool == "o" else (ps_b, "psb")
                fo = pl.tile([128, 1024], F32, tag=tg,
                             name=f"fin{r}_{qt_i}")
                for nb in range(2):
                    for hdt in range(2):
                        nc.tensor.matmul(
                            fo[:, ts(nb, 512)],
                            st.OT[hdt][:, ts(qt_i, 128)],
                            st.wo_sb[hdt][:, ts(nb, 512)],
                            start=(hdt == 0), stop=(hdt == 1))
                if split:
                    for nb in range(2):
                        fs = p_fo.tile([128, 512], F32, tag="fo2",
                                       name=f"fs{r}_{qt_i}_{nb}")
                        if nb == 0:
                            nc.scalar.activation(fs, fo[:, ts(nb, 512)], COPY)
                        else:
                            nc.vector.tensor_copy(fs, fo[:, ts(nb, 512)])
                        nc.sync.dma_start(
                            out=out[ts(qt_i, 128), ts(nb, 512)], in_=fs)
                else:
                    fs = p_fo.tile([128, 1024], F32, tag="fo",
                                   name=f"fs{r}_{qt_i}")
                    nc.vector.tensor_copy(fs, fo)
                    nc.sync.dma_start(out=out[ts(qt_i, 128), :], in_=fs)
            return f

        # ---- attention block machinery -----------------------------------
        def normalize(r, otp, qh, g, h, qq):
            st = reps[r]
            rc = p_rc.tile([64, 512], F32, tag="rc", name=f"rc{r}_{qh}{g}{h}{qq}")
            nc.vector.reciprocal(rc, otp[ds(64, 64), ts(h, 512)])
            nc.vector.tensor_mul(
                st.OT[g][ds(h * 64, 64), ds(qh * 1024 + qq * 512, 512)],
                otp[ds(0, 64), ts(h, 512)], rc)

        def run_block(r, qh, g, extras, lag=PV_LAG):
            st = reps[r]
            if st.OT[g] is None:
                st.OT[g] = p_ot.tile([128, S], MMDT, tag="ot",
                                     name=f"OT{r}_{g}")
            es_store = {}
            otp_store = {}

            def pv_work(kt, qq):
                def f():
                    if kt == 0:
                        otp_store[qq] = ps_o.tile(
                            [128, 1024], F32, tag="otp",
                            name=f"otp{r}_{qh}{g}{qq}")
                    otp = otp_store[qq]
                    es = es_store.pop((qq, kt))
                    for h in range(2):
                        nc.tensor.matmul(
                            otp[:, ts(h, 512)],
                            st.VA[kt][:, ds((g * 2 + h) * 128, 128)],
                            es[:, ts(h, 512)],
                            start=(kt == 0), stop=(kt == 15))
                    if kt == 15:
                        for h in range(2):
                            normalize(r, otp, qh, g, h, qq)
                return f

            order = [(kt, 0) for kt in range(16)] + [(kt, 1) for kt in range(16)]
            for i, (kt, qq) in enumerate(order):
                while len(st.pv_queue) > lag:
                    st.pv_queue.pop(0)()
                if i < len(extras) and extras[i] is not None:
                    extras[i]()
                sc = ps_b.tile([128, 1024], F32, tag="psb",
                               name=f"sc{r}_{qh}{g}{kt}{qq}")
                for h in range(2):
                    nc.tensor.matmul(
                        sc[:, ts(h, 512)],
                        st.KT[g][kt // 4][ds(h * 64, 64), ts(kt % 4, 128)],
                        st.QT[g][qh * 2 + qq][ds(h * 64, 64), :],
                        start=True, stop=True)
                es = p_es.tile([128, 1024], BF16, tag="es",
                               name=f"es{r}_{qh}{g}{kt}{qq}")
                nc.scalar.activation(es, sc, EXP, scale=0.125)
                es_store[(qq, kt)] = es
                st.pv_queue.append(pv_work(kt, qq))
            for i in range(len(order), len(extras)):
                if extras[i] is not None:
                    extras[i]()

        def drain_pv(r):
            st = reps[r]
            while st.pv_queue:
                st.pv_queue.pop(0)()

        # ---- schedule ----------------------------------------------------
        # Steady state: rep r's g0 K/Q projections run inside rep r-1's C/D
        # blocks (their QT/KT pool slots free mid-rep), v(0..2) in rep r-1's
        # D tail. Fin pieces follow their OT-normalize availability: 0-3
        # late-B, 4-7 early-C, 8-11 late-D, 12-15 at the boundary.
        def placed(pairs, n=32):
            ex = [None] * n
            for i, p in pairs:
                ex[i] = p
            return ex

        dmas(0)
        kq_piece(0, 0, 0, 0)()
        kq_piece(0, 0, 0, 1)()

        for r in range(repeat):
            kq = lambda g, qb, w: kq_piece(r, g, qb, w)
            v = lambda stt: v_piece(r, stt)
            nxt = r + 1 if r + 1 < repeat else None

            if r == 0:
                extrasA = [
                    v(0), kq(0, 1, 0), v(1), v(2), v(3), kq(0, 2, 0),
                    v(4), v(5), v(6), kq(0, 3, 0), v(7), v(8), v(9),
                    kq(0, 1, 1), v(10), v(11), v(12), v(13), v(14), v(15),
                    kq(1, 0, 0), kq(1, 0, 1),
                ]
            else:
                extrasA = placed([
                    (0, v(3)), (2, v(4)), (4, v(5)), (6, v(6)), (8, v(7)),
                    (10, v(8)), (12, v(9)), (14, v(10)), (16, v(11)),
                    (17, v(12)), (18, v(13)), (19, v(14)), (20, v(15)),
                    (22, kq(1, 0, 0)), (25, kq(1, 0, 1)),
                ])
            run_block(r, 0, 0, extrasA)

            extrasB = placed([
                (1, kq(1, 1, 0)), (3, kq(0, 2, 1)), (5, kq(1, 2, 0)),
                (7, kq(0, 3, 1)), (9, kq(1, 3, 0)), (13, kq(1, 1, 1)),
                (20, fin_piece(r, 0)), (22, fin_piece(r, 1)),
                (25, fin_piece(r, 2)), (28, fin_piece(r, 3)),
            ])
            run_block(r, 0, 1, extrasB, lag=4)

            if nxt is not None:
                dmas(nxt)

            pairsC = [
                (5, fin_piece(r, 4)), (7, fin_piece(r, 5)),
                (9, fin_piece(r, 6)), (11, fin_piece(r, 7)),
                (13, kq(1, 2, 1)), (16, kq(1, 3, 1)),
            ]
            if nxt is not None:
                pairsC += [(19, kq_piece(nxt, 0, 0, 1)),
                           (22, kq_piece(nxt, 0, 1, 1))]
            run_block(r, 1, 0, placed(pairsC), lag=4)

            pairsD = [
                (19, fin_piece(r, 8)), (21, fin_piece(r, 9)),
                (23, fin_piece(r, 10)), (25, fin_piece(r, 11)),
            ]
            if nxt is not None:
                # v(nxt, st) must trail D's (qq1, kt=st) PV pop (slot 20+st)
                pairsD = [
                    (0, kq_piece(nxt, 0, 0, 0)), (3, kq_piece(nxt, 0, 1, 0)),
                    (6, kq_piece(nxt, 0, 2, 0)), (9, kq_piece(nxt, 0, 3, 0)),
                    (19, fin_piece(r, 8)), (20, v_piece(nxt, 0)),
                    (21, fin_piece(r, 9)), (22, v_piece(nxt, 1)),
                    (23, fin_piece(r, 10)), (24, v_piece(nxt, 2)),
                    (25, fin_piece(r, 11)),
                ]
            run_block(r, 1, 1, placed(pairsD), lag=3)
            drain_pv(r)

            for q8 in range(12, 16):
                fin_piece(r, q8, pool="o", split=True)()

    nc.compile()
    return nc


# ---------------- host-side helpers ----------------


def rel_pos_enc(seq_len, dim):
    positions = np.arange(seq_len, dtype=np.float32)[:, None]
    div_term = np.exp(
        np.arange(0, dim, 2, dtype=np.float32) * (-(np.log(10000.0) / dim))
    )
    pe = np.zeros((seq_len, dim), dtype=np.float32)
    pe[:, 0::2] = np.sin(positions * div_term)
    pe[:, 1::2] = np.cos(positions * div_term)
    return pe


def core_inputs(x, W_qkv, b_qkv, core):
    b = core // 4
    h0 = (core % 4) * 4
    cols = slice(h0 * 64, (h0 + 4) * 64)
    xt = np.ascontiguousarray(x[b].T)
    wq = np.ascontiguousarray(W_qkv[:, 0:1024][:, cols])
    wk = np.ascontiguousarray(W_qkv[:, 1024:2048][:, cols])
    wv = np.ascontiguousarray(W_qkv[:, 2048:3072][:, cols])
    bq = b_qkv[0:1024][cols]
    bk = b_qkv[1024:2048][cols]
    bqk = np.stack(
        [bq[0:128], bq[128:256], bk[0:128], bk[128:256]], axis=1
    ).astype(np.float32)
    pos = rel_pos_enc(S, HD)  # [S, 64]
    post = np.ascontiguousarray(
        np.concatenate([pos.T, pos.T], axis=0).astype(np.float32)
    )  # [128, S]
    return {
        "xt": xt,
        "wq": wq,
        "wk": wk,
        "wv": wv,
        "bqk": np.ascontiguousarray(bqk),
        "post": post,
    }


def core_inputs_out(W_out, core):
    h0 = (core % 4) * 4
    rows = slice(h0 * 64, (h0 + 4) * 64)
    return {"wo": np.ascontiguousarray(W_out[rows, :])}


def all_core_inputs(x, W_qkv, b_qkv, W_out):
    ins = []
    for c in range(8):
        m = core_inputs(x, W_qkv, b_qkv, c)
        m.update(core_inputs_out(W_out, c))
        ins.append(m)
    return ins


def combine_outputs(partials, b_qkv, W_out, b_out):
    extra = b_qkv[2048:3072] @ W_out + b_out  # [DM]
    outs = []
    for b in range(2):
        acc = partials[b * 4].astype(np.float64)
        for c in range(b * 4 + 1, b * 4 + 4):
            acc = acc + partials[c]
        outs.append((acc + extra).astype(np.float32))
    return np.stack(outs, axis=0)  # [2, S, DM]


_CACHE = {}


def _get_program():
    if "nc" not in _CACHE:
        _CACHE["nc"] = build_program(use_f32r=True)
    return _CACHE["nc"]


def kernel(x, W_qkv, b_qkv, W_out, b_out):
    x = np.ascontiguousarray(np.asarray(x, dtype=np.float32))
    W_qkv = np.ascontiguousarray(np.asarray(W_qkv, dtype=np.float32))
    b_qkv = np.asarray(b_qkv, dtype=np.float32)
    W_out = np.ascontiguousarray(np.asarray(W_out, dtype=np.float32))
    b_out = np.asarray(b_out, dtype=np.float32)

    from concourse import bass_utils

    nc = _get_program()
    in_maps = all_core_inputs(x, W_qkv, b_qkv, W_out)
    res = bass_utils.run_bass_kernel_spmd(nc, in_maps, core_ids=list(range(8)))
    partials = [res.results[c]["out"] for c in range(8)]
    return combine_outputs(partials, b_qkv, W_out, b_out)
